# revision 40
# baseline (speedup 1.0000x reference)
"""Trainium2 Bass kernel for the KerasArima 2nd-order linear recurrence.

Reference computes, per lane (b, h, w):
    y_t = x_t + phi*(x_t - x_{t-1}) - theta_1*(x_t - y_{t-1}) - theta_2*(x_{t-1} - y_{t-2})
which is a linear constant-coefficient recurrence
    y_t = a*x_t + b*x_{t-1} + c*y_{t-1} + d*y_{t-2}
with a = 1+phi-theta_1, b = -(phi+theta_2), c = theta_1, d = theta_2.
Because |c|,|d| < 0.2 the impulse response g decays below fp32 eps within ~45
lags, so y is exactly (to fp32) a short causal convolution of x. Blocked into
128-step time blocks this becomes two dense 128x128 Toeplitz matmuls per block.

To cut HBM traffic the device computes the *residual* d = y - x (drop the
identity from the diagonal Toeplitz block): d has std ~0.25 vs y's ~1.45, so
it quantizes to int8 with an 8-sigma clip at ~2e-3 relative error. The input
is ALSO int8 (host scales x by 127/max|x|; the quantization error only enters
y through the small residual filter since the host adds the exact fp32 x
back). Per core that is 4.2 MB in + 2.1 MB out instead of 33.5 MB fp32.

Device datapath: int8 loads ride a casting SWDGE DMA (int8 DRAM -> fp16 SBUF
inline, integers exact in fp16), fp16 matmuls (PE at 1 cycle/row), PSUM fp32,
PSUM->SBUF evacuation split across DVE and ACT with the combined dequant/quant
scale fused, int8 stores on the sync HWDGE ring (separate from the load ring
so store completion never blocks load prefetch). Host pre-transposes x to
[batch, t%128, t//128, lane] so every DMA moves >=2KB contiguous runs per
partition, and applies the first-block initial-condition correction (rank-1
in x_0 plus an e_0 bias) at the end.

Sharding: pure data parallelism - batch axis split 8 ways across NeuronCores.
"""

import numpy as np

# Problem shape (hardcoded per contract)
B, T, H, W = 64, 2048, 16, 16
LANES = H * W                # 256
NCORES = 8
BPC = B // NCORES            # 8 batches per core
P = 128                      # time-block size = partition count
NBLK = T // P                # 16 blocks per batch
PAIR = 2                     # batches fused into one matmul free dim
FREE = PAIR * LANES          # 512 = one PSUM bank of fp32
NPAIR = BPC // PAIR          # 4
CHUNK = 8                    # time blocks per PSUM-bank group

_cache = {}


def _coeffs(phi, t1, t2, e0):
    """Host-side (float64) Toeplitz blocks + first-block corrections.

    Returns (M0d^T fp16, M1^T fp16, q, r, sd) where the device computes
    D_blk = (M0 - I) @ X_blk + M1 @ X_{blk-1}  (zero-state residual d = y - x),
    the host fixes up block 0 with y[:, t] += q[t]*x[:, 0] + r[t], and sd is
    the residual's std per unit x std (for the int8 scale).
    """
    a = 1.0 + phi - t1
    b = -(phi + t2)
    c, d = t1, t2
    K = 2 * P
    h = np.zeros(K + 1)
    h[0] = 1.0
    h[1] = c
    for k in range(2, K + 1):
        h[k] = c * h[k - 1] + d * h[k - 2]
    g = np.zeros(K + 1)
    g[0] = a
    g[1:] = a * h[1:] + b * h[:-1]

    M0 = np.zeros((P, P))
    for j in range(P):
        M0[j:, j] = g[:P - j]
    np.fill_diagonal(M0, M0.diagonal() - 1.0)   # residual: drop identity
    M1 = np.zeros((P, P))
    for j in range(P):
        M1[:, j] = g[P - j:2 * P - j]

    # Initial-condition corrections (zero-state -> true y_0, y_1):
    #   delta0 = (t1-phi)*x_0 - t1*e0 ; delta1 = t2*(x_0 - e0)
    # y_t += h_t*delta0 + h_{t-1}*delta1  for t in [0, P)
    q = np.zeros(P)
    r = np.zeros(P)
    q[0] = t1 - phi
    r[0] = -e0 * t1
    q[1:] = (t1 - phi) * h[1:P] + t2 * h[:P - 1]
    r[1:] = -e0 * (t1 * h[1:P] + t2 * h[:P - 1])

    gd = g.copy()
    gd[0] -= 1.0
    sd = float(np.sqrt(np.sum(gd ** 2)))
    return (
        np.ascontiguousarray(M0.T).astype(np.float16),
        np.ascontiguousarray(M1.T).astype(np.float16),
        q,
        r,
        sd,
    )


def _build(reps=1, scale=None):
    """Build + compile the single-core Bass program (same program on all cores).

    scale: fp32 multiplier fused into the PSUM->SBUF evacuation (int8 quant).
    """
    import concourse.bacc as bacc
    import concourse.mybir as mybir
    import concourse.tile as tile

    if scale is None:
        scale = _cache.get("scale", 64.0)
    F16 = mybir.dt.float16
    F32 = mybir.dt.float32
    I8 = mybir.dt.int8
    HALF = NBLK // 2

    nc = bacc.Bacc(trn_type="TRN2", target_bir_lowering=False, debug=False)

    # x/y are host-pretransposed: [batch, t%P, t//P, lane] so each per-batch
    # DMA is partition-major with contiguous >=2KB runs per partition.
    x = nc.dram_tensor("x", [BPC, P, NBLK, LANES], I8, kind="ExternalInput").ap()
    w0 = nc.dram_tensor("w0", [P, P], F16, kind="ExternalInput").ap()
    w1 = nc.dram_tensor("w1", [P, P], F16, kind="ExternalInput").ap()
    y = nc.dram_tensor("y", [BPC, P, NBLK, LANES], I8, kind="ExternalOutput").ap()

    with tile.TileContext(nc) as tc:
        with tc.tile_pool(name="const", bufs=1) as cpool, \
             tc.tile_pool(name="xin", bufs=4) as xpool, \
             tc.tile_pool(name="yout", bufs=4) as ypool, \
             tc.tile_pool(name="ps", bufs=8, space="PSUM") as ppool:

            w0t = cpool.tile([P, P], F16)
            w1t = cpool.tile([P, P], F16)
            # weights on the HWDGE ring; they land during the preamble
            nc.sync.dma_start(out=w0t[:], in_=w0[:])
            nc.sync.dma_start(out=w1t[:], in_=w1[:])

            def body(_=None):
                for pair in range(NPAIR):
                    b0 = pair * PAIR
                    xt = xpool.tile([P, PAIR, NBLK, LANES], F16)
                    # casting loads (int8 DRAM -> fp16 SBUF) on SWDGE. First
                    # pair split in halves so chunk-0 compute starts after two
                    # 256KB transfers (subtile deps); later pairs move both
                    # batches in one transfer.
                    if pair == 0:
                        # graded lead-in: the k=0 matmul only needs block 0,
                        # so a small first transfer unblocks compute earliest
                        for ks in (slice(0, 4), slice(4, 8), slice(8, NBLK)):
                            for bb in range(PAIR):
                                nc.gpsimd.dma_start(out=xt[:, bb, ks],
                                                    in_=x[b0 + bb, :, ks])
                    else:
                        for bb in range(PAIR):
                            nc.gpsimd.dma_start(out=xt[:, bb],
                                                in_=x[b0 + bb])
                    ot = ypool.tile([P, PAIR, NBLK, LANES], I8)
                    for k0 in range(0, NBLK, CHUNK):
                        pts = [ppool.tile([P, FREE], F32, name="pt", tag="pt")
                               for _ in range(CHUNK)]
                        # all M0 matmuls of the chunk share stationary w0,
                        # then all M1 matmuls share stationary w1
                        for i, k in enumerate(range(k0, k0 + CHUNK)):
                            nc.tensor.matmul(pts[i][:], w0t[:],
                                             xt[:, :, k, :],
                                             start=True, stop=(k == 0))
                        for i, k in enumerate(range(k0, k0 + CHUNK)):
                            if k == 0:
                                continue
                            nc.tensor.matmul(pts[i][:], w1t[:],
                                             xt[:, :, k - 1, :],
                                             start=False, stop=True)
                        # evacuate PSUM with the int8 quant fused, split
                        # across DVE and ACT (odd blocks -> DVE so the last
                        # block's evacuation rides the faster engine)
                        for i, k in enumerate(range(k0, k0 + CHUNK)):
                            if k % 2 == 1:
                                nc.vector.tensor_scalar_mul(
                                    out=ot[:, :, k, :], in0=pts[i][:],
                                    scalar1=float(scale))
                            else:
                                nc.scalar.activation(
                                    ot[:, :, k, :], pts[i][:],
                                    mybir.ActivationFunctionType.Copy,
                                    scale=float(scale))
                    # stores on the sync HWDGE ring (separate from the SWDGE
                    # load ring). Last pair split in halves so the final
                    # store only waits on the last chunk's evacuation; earlier
                    # pairs use full-batch transfers.
                    if pair == NPAIR - 1:
                        # graded tail-out: shrink the last store so it waits
                        # on as little evacuation as possible, and put the
                        # final two transfers on separate rings (the SWDGE
                        # load ring is idle by now)
                        for ks in (slice(0, HALF), slice(HALF, 12)):
                            for bb in range(PAIR):
                                nc.sync.dma_start(out=y[b0 + bb, :, ks],
                                                  in_=ot[:, bb, ks])
                        ks = slice(12, NBLK)
                        nc.sync.dma_start(out=y[b0, :, ks], in_=ot[:, 0, ks])
                        nc.gpsimd.dma_start(out=y[b0 + 1, :, ks],
                                            in_=ot[:, 1, ks])
                    else:
                        for bb in range(PAIR):
                            nc.sync.dma_start(out=y[b0 + bb], in_=ot[:, bb])

            if reps == 1:
                body()
            else:
                with tc.For_i(0, reps, 1) as _i:
                    body()

    nc.compile()
    return nc


def _in_maps(x, phi, theta_1, theta_2, e_0):
    w0, w1, q, r, sd = _coeffs(float(phi[0]), float(theta_1[0]),
                               float(theta_2[0]), float(e_0[0]))
    xs = np.asarray(x, np.float32).reshape(B, NBLK, P, LANES)
    x_std = float(xs.std())
    maxabs = float(np.abs(xs).max())
    sx = max(maxabs, 1e-6) / 127.0          # input dequant step
    clip_d = 8.0 * sd * max(x_std, 1e-6)    # residual clip (8 sigma)
    # psum = d / sx, so one fused evacuation multiplier quantizes to int8:
    _cache["qr"] = (q, r)
    _cache["scale"] = sx * 127.0 / clip_d
    _cache["oscale"] = clip_d / 127.0       # host-side output dequant step
    xs = xs.transpose(0, 2, 1, 3)           # [B, P, NBLK, LANES]
    xs = np.rint(xs * (1.0 / sx)).astype(np.int8)
    xs = xs.reshape(NCORES, BPC, P, NBLK, LANES)
    return [
        {"x": xs[i], "w0": w0, "w1": w1}
        for i in range(NCORES)
    ]


def kernel(x, phi, theta_1, theta_2, e_0):
    from concourse.bass_utils import run_bass_kernel_spmd

    in_maps = _in_maps(x, phi, theta_1, theta_2, e_0)
    scale = _cache["scale"]
    key = ("nc", round(scale, 6))
    if key not in _cache:
        _cache[key] = _build(reps=1, scale=scale)
    nc = _cache[key]
    res = run_bass_kernel_spmd(nc, in_maps, core_ids=list(range(NCORES)))
    dt = np.stack([res.results[i]["y"] for i in range(NCORES)])
    dt = dt.reshape(B, P, NBLK, LANES).astype(np.float32) * np.float32(_cache["oscale"])
    dres = dt.transpose(0, 2, 1, 3).reshape(B, T, H, W)
    y = np.asarray(x, np.float32) + dres
    # first-block initial-condition correction (rank-1 in x_0 plus bias)
    q, r = _cache["qr"]
    x0 = np.asarray(x, np.float32)[:, 0].reshape(B, LANES)
    corr = q[None, :, None] * x0[:, None, :].astype(np.float64) + r[None, :, None]
    y[:, :P] += corr.astype(np.float32).reshape(B, P, H, W)
    return y.astype(np.float32)
